# revision 48
# baseline (speedup 1.0000x reference)
"""Gated linear attention (GLA) Bass kernel for Trainium2, 8 NeuronCores.

Sharding: one core per (batch, head) pair -- B=2 x H=4 = 8 cores.

v2 redesign vs baseline (145us):
  - bf16 throughout: x, weights, chunk matmuls (4x PE rate at 128-col moving
    dim; FWL weight loads; half the HBM traffic for x).
  - transposed-o formulation: po^T[dv,t] = v^T atm + S^T q~ accumulates both
    recurrence terms in one psum group, killing the per-chunk output
    transpose + copy of the baseline.
  - phases pipelined across chunks instead of a serial per-chunk chain:
    B1 (pat/ktn/pds/po-intra, independent per chunk), B2 (state scan
    U_c = U_{c-1}*d_{c-1} + pds_c as one fused vector op reading psum),
    B3 (po += S^T q~), C (rmsnorm via matmul-with-ones + batched Ln/Exp).
  - rmsnorm rstd applied on the 10-col head output instead of the dv-wide
    tensor; sum-of-squares via PE matmul against ones.
  - decay exp computed as ONE [128,512] activation (q|k halves stacked with
    pre-negated sign); softmax scale folded into Wq on host.
  - swish split across scalar/gpsimd/vector; cumsum j>0 on gpsimd.
  - x streamed t-major [128,T,4] so time slices are contiguous DMAs;
    weights packed into 2 DMAs ordered ahead of the x stream.
"""
import sys, os
sys.path.insert(0, "/opt/trn_rl_repo")

import numpy as np

B, T, D = 2, 2048, 512
H = 4
dk, dv = 64, 128          # per-head key/value dims
C = 128                   # chunk length
GATE_NORM = 16.0
EPS = 1e-5
SCALE = dk ** -0.5

_CACHE = {}


def build(t=T):
    import concourse.bass as bass  # noqa: F401
    from concourse import bacc, mybir
    import concourse.tile as tile
    import concourse.hw_specs as hw_specs

    F32 = mybir.dt.float32
    BF16 = mybir.dt.bfloat16
    AF = mybir.ActivationFunctionType
    OP = mybir.AluOpType

    # Keep every activation func we use in one table (see baseline comment):
    # Exp, Ln, Square, Copy, Identity all live in natural_log_exp_and_others.
    need = {AF.Exp, AF.Ln, AF.Square, AF.Copy, AF.Identity}
    keep = "natural_log_exp_and_others"
    tabs = hw_specs.get_activation_tables("gen3")
    if keep in tabs and need <= tabs[keep]:
        for name, s in tabs.items():
            if name != keep:
                s -= need

    nch = t // C              # chunks
    nts = t // 512            # 512-wide time slices
    assert t % 512 == 0 and nch % 4 == 0
    ngrp = nch // 4

    nc = bacc.Bacc("TRN2", target_bir_lowering=False, debug=False)

    xt_d = nc.dram_tensor("xt", [128, 4, t], BF16, kind="ExternalInput")
    w1_d = nc.dram_tensor("w1", [128, 4, dk], BF16, kind="ExternalInput")
    w2_d = nc.dram_tensor("w2", [128, 4, 2 * C + dv], BF16, kind="ExternalInput")
    um_d = nc.dram_tensor("umask", [C, C], BF16, kind="ExternalInput")
    wf_d = nc.dram_tensor("wfused", [dv, 10], BF16, kind="ExternalInput")
    nb_d = nc.dram_tensor("nbgk2", [dk, 1], F32, kind="ExternalInput")
    out_d = nc.dram_tensor("out10", [128, nch, 10], F32, kind="ExternalOutput")

    with tile.TileContext(nc) as tc:
        with (
            tc.tile_pool(name="wt", bufs=1) as wt,
            tc.tile_pool(name="sm", bufs=2) as sm,
            tc.tile_pool(name="ck", bufs=3) as ck,
            tc.tile_pool(name="am", bufs=8) as am,
            tc.tile_pool(name="pp", bufs=4, space="PSUM") as pp,
            tc.tile_pool(name="pq", bufs=1, space="PSUM") as pq,
        ):
            # ---- persistent SBUF ----
            xt = wt.tile([128, 4, t], BF16)
            w1 = wt.tile([128, 4, dk], BF16)           # wgk (low-rank fused)
            w2 = wt.tile([128, 4, 2 * C + dv], BF16)   # [wqk 128 | wg 128 | wv 128]
            umb = wt.tile([C, C], BF16)
            wf_sb = wt.tile([dv, 10], BF16)
            nb_sb = wt.tile([dk, 1], F32)

            # input DMAs: weights ahead of the x stream on the sync queue.
            # first 512-slice split into 4 pieces so slice-0 projections can
            # start on partial data.
            nc.sync.dma_start(w1[:], w1_d[:])
            for p in range(2):
                nc.sync.dma_start(xt[:, :, p * 256:(p + 1) * 256],
                                  xt_d[:, :, p * 256:(p + 1) * 256])
            nc.sync.dma_start(w2[:], w2_d[:])
            for j in range(1, nts):
                nc.sync.dma_start(xt[:, :, j * 512:(j + 1) * 512],
                                  xt_d[:, :, j * 512:(j + 1) * 512])
            nc.gpsimd.dma_start(umb[:], um_d[:])
            nc.gpsimd.dma_start(wf_sb[:], wf_d[:])
            nc.gpsimd.dma_start(nb_sb[:], nb_d[:])

            wqk = w2[:, :, 0:C]
            wg = w2[:, :, C:2 * C]
            wv = w2[:, :, 2 * C:2 * C + dv]

            ones64 = wt.tile([dk, 1], F32)
            nc.vector.memset(ones64[:], 1.0)
            onesbf = wt.tile([128, 1], BF16)
            nc.vector.memset(onesbf[:], 1.0)
            eps_sb = wt.tile([128, 1], F32)
            nc.vector.memset(eps_sb[:], EPS)
            # scan reset mask: 0 at chunk starts
            mres = wt.tile([dk, 512], F32)
            nc.vector.memset(mres[:], 1.0)
            mres_v = mres[:].rearrange("p (c l) -> p c l", l=C)
            nc.vector.memset(mres_v[:, :, 0:1], 0.0)

            # big SBUF activations
            spc = wt.tile([dk, t], F32)       # per-chunk cumsum of softplus
            qt_t = wt.tile([dk, t], BF16)     # q~^T
            kt_t = wt.tile([dk, t], BF16)     # k~^T
            swt = wt.tile([dv, t], BF16)      # swish(g)^T
            vsb = wt.tile([128, nch, dv], BF16)
            dlast = wt.tile([dk, nch], F32)
            ktn = wt.tile([C, nch, dk], BF16)
            Sb = wt.tile([dk, nch, dv], BF16)
            obuf = wt.tile([128, nch, 10], F32)

            dl_src = spc[:].rearrange("p (c l) -> p c l", l=C)

            # ---- PSUM (bank-granular: 4 + 1 + 1 + 1 = 7 of 8 banks) ----
            prs = pq.tile([128, 512], F32)    # pat 2 slots | pds 2 slots
            pvt = pq.tile([128, 4, dv], F32)  # v projection, 4 rotating slots
            pot = pq.tile([128, 4, C], F32)   # po^T, 4 rotating slots
            # ssq cols | p10 cols | 2 odd-po slots
            pt2 = pq.tile([128, nch + nch * 10 + 2 * C], F32)

            def pat_s(c):
                return prs[:, (c % 2) * C:(c % 2) * C + C]

            def pds_s(c):
                return prs[0:dk, 256 + (c % 2) * C:256 + (c % 2) * C + C]

            def ssq_s(c):
                return pt2[:, c:c + 1]

            def p10_s(c):
                return pt2[:, nch + c * 10:nch + (c + 1) * 10]

            def sl(c):
                return slice(c * C, (c + 1) * C)

            # ---------------- phase A: projections ----------------
            def emit_A(j):
                ts = slice(j * 512, (j + 1) * 512)
                xs = xt[:, :, ts]
                # j=0 runs piecewise so matmuls start as soon as the first
                # 256-col DMA piece lands
                pieces = ([slice(p * 256, (p + 1) * 256) for p in range(2)]
                          if j == 0 else [slice(0, 512)])

                def proj(ps, w_sb):
                    for pr in pieces:
                        for d4 in range(4):
                            nc.tensor.matmul(ps[:, pr], w_sb[:, d4, :],
                                             xs[:, d4, pr],
                                             start=(d4 == 0), stop=(d4 == 3))

                # gate chain
                pz = pp.tile([dk, 512], F32, tag="P")
                proj(pz, w1)
                eg = sm.tile([dk, 512], BF16, tag="eg")
                nc.scalar.activation(out=eg[:], in_=pz[:], func=AF.Exp,
                                     scale=-1.0, bias=nb_sb[:])
                sp = sm.tile([dk, 512], F32, tag="sp")
                nc.scalar.activation(out=sp[:], in_=eg[:], func=AF.Ln,
                                     bias=ones64[:])
                nc.vector.tensor_tensor_scan(
                    out=spc[:, ts], data0=mres[:], data1=sp[:],
                    initial=0.0, op0=OP.mult, op1=OP.add)
                nc.scalar.activation(out=dlast[:, 4 * j:4 * j + 4],
                                     in_=dl_src[:, 4 * j:4 * j + 4, C - 1:C],
                                     func=AF.Exp, scale=-1.0 / GATE_NORM)
                eeq = sm.tile([dk, 512], F32, tag="eeq")
                nc.scalar.activation(out=eeq[:], in_=spc[:, ts], func=AF.Exp,
                                     scale=-1.0 / GATE_NORM)
                eek = sm.tile([dk, 512], F32, tag="eek")
                nc.scalar.activation(out=eek[:], in_=spc[:, ts], func=AF.Exp,
                                     scale=1.0 / GATE_NORM)

                # q|k projection + decay
                pqk = pp.tile([128, 512], F32, tag="P")
                proj(pqk, wqk)
                nc.vector.tensor_tensor(out=qt_t[:, ts], in0=pqk[0:dk, :],
                                        in1=eeq[:], op=OP.mult)
                nc.vector.tensor_tensor(out=kt_t[:, ts], in0=pqk[64:128, :],
                                        in1=eek[:], op=OP.mult)
                # k^T chunk transposes on the (idle) DMA xbar: [64,512] ->
                # [128, 4, 64] blocked per chunk, ahead of the phase-B scan
                nc.sync.dma_start_transpose(ktn[:, 4 * j:4 * j + 4, :],
                                            kt_t[:, ts])


                # g^T projection + swish
                pgt = pp.tile([128, 512], F32, tag="P")
                proj(pgt, wg)
                eg2 = sm.tile([dv, 512], BF16, tag="eg2")
                nc.scalar.activation(out=eg2[:], in_=pgt[:], func=AF.Exp,
                                     scale=-1.0)
                s1 = sm.tile([dv, 512], F32, tag="s1")
                nc.vector.tensor_scalar_add(out=s1[:], in0=eg2[:], scalar1=1.0)
                s2 = sm.tile([dv, 512], F32, tag="s2")
                nc.vector.reciprocal_approx_fast(out=s2[:], in_=s1[:])
                nc.vector.tensor_tensor(out=swt[:, ts], in0=pgt[:], in1=s2[:],
                                        op=OP.mult)

                # v natural projections
                for i in range(4):
                    tt = 4 * j + i
                    pvs = pvt[:, tt % 4, :]
                    for d4 in range(4):
                        nc.tensor.matmul(pvs, xs[:, d4, i * C:(i + 1) * C],
                                         wv[:, d4, :],
                                         start=(d4 == 0), stop=(d4 == 3))
                    if i % 2 == 0:
                        nc.scalar.copy(vsb[:, tt, :], pvs)
                    else:
                        nc.vector.tensor_copy(vsb[:, tt, :], pvs)

            # ---------------- phase B/C: chunked recurrence ----------------
            U = [None, None]

            def po_s(c):
                # alternate po groups between two banks so consecutive
                # accumulation groups never contend for the same psum bank
                if c % 2 == 0:
                    return pot[:, (c // 2) % 4, :]
                base = nch + nch * 10 + ((c // 2) % 2) * C
                return pt2[:, base:base + C]

            def emit_c(c):
                """post-processing of chunk c (po complete)."""
                po = po_s(c)
                ot = ck.tile([dv, C], BF16, tag="ot")
                nc.vector.tensor_tensor(out=ot[:], in0=po, in1=swt[:, sl(c)],
                                        op=OP.mult)
                sq = ck.tile([dv, C], BF16, tag="sq")
                nc.scalar.activation(out=sq[:], in_=po, func=AF.Square)
                nc.tensor.matmul(ssq_s(c), sq[:], onesbf[:],
                                 start=True, stop=True)
                nc.tensor.matmul(p10_s(c), ot[:], wf_sb[:],
                                 start=True, stop=True)
                if c % 4 == 3:
                    g = c // 4
                    lnv = ck.tile([128, 4], F32, tag="lnv")
                    nc.scalar.activation(out=lnv[:], in_=pt2[:, 4 * g:4 * g + 4],
                                         func=AF.Ln, scale=1.0 / dv,
                                         bias=eps_sb[:])
                    rstd = ck.tile([128, 4], F32, tag="rstd")
                    nc.scalar.activation(out=rstd[:], in_=lnv[:], func=AF.Exp,
                                         scale=-0.5)
                    p10g = pt2[:, nch + g * 40:nch + (g + 1) * 40]
                    nc.vector.tensor_tensor(
                        out=obuf[:, 4 * g:4 * g + 4, :],
                        in0=p10g.rearrange("p (c n) -> p c n", n=10),
                        in1=rstd[:].unsqueeze(2).broadcast_to([128, 4, 10]),
                        op=OP.mult)
                    nc.sync.dma_start(out_d[:, 4 * g:4 * g + 4, :],
                                      obuf[:, 4 * g:4 * g + 4, :])

            LAG = min(6, nch - 1)
            atms = [None] * nch

            def emit_tail(x):
                po = po_s(x)
                if x > 0:
                    nc.tensor.matmul(po, Sb[:, x - 1, :], qt_t[:, sl(x)],
                                     start=True, stop=False)
                nc.tensor.matmul(po, vsb[:, x, :], atms[x][:],
                                 start=(x == 0), stop=True)
                emit_c(x)

            def emit_B(c):
                cs = sl(c)
                qt_c = qt_t[:, cs]
                kt_c = kt_t[:, cs]
                v_c = vsb[:, c, :]
                # B1
                pat = pat_s(c)
                nc.tensor.matmul(pat, kt_c, qt_c, start=True, stop=True)
                # pat evict on scalar, causal mask on gpsimd - keeps the
                # loaded vector queue out of this path entirely
                patr = am.tile([C, C], BF16, tag="patr")
                if c % 2 == 0:
                    nc.scalar.copy(patr[:], pat)
                else:
                    nc.vector.tensor_copy(patr[:], pat)
                atm = am.tile([C, C], BF16, tag="atm")
                atms[c] = atm
                nc.gpsimd.tensor_tensor(out=atm[:], in0=patr[:], in1=umb[:],
                                        op=OP.mult)
                pds = pds_s(c)
                nc.tensor.matmul(pds, ktn[:, c, :], v_c, start=True, stop=True)
                # B2: U_c = U_{c-1} * d_{c-1} + pds_c ; Sb_c = bf16(U_c * d_c)
                Uc = ck.tile([dk, dv], F32, tag="U")
                if c == 0:
                    nc.vector.tensor_copy(Uc[:], pds)
                else:
                    nc.vector.scalar_tensor_tensor(
                        out=Uc[:], in0=U[(c - 1) % 2][:],
                        scalar=dlast[:, c - 1:c], op0=OP.mult,
                        in1=pds, op1=OP.add)
                U[c % 2] = Uc
                nc.gpsimd.tensor_tensor(
                    out=Sb[:, c, :], in0=Uc[:],
                    in1=dlast[:, c:c + 1].broadcast_to([dk, dv]), op=OP.mult)

            # ------------- interleaved schedule -------------
            # A-slices and B-chunk-groups interleave so phase-B work fills
            # phase-A queue gaps; chunk tails trail by LAG behind the scan.
            ntail = [0]

            def emit_B_group(c0):
                for c in range(c0, c0 + 4):
                    emit_B(c)
                    while ntail[0] <= c - LAG:
                        emit_tail(ntail[0])
                        ntail[0] += 1

            emit_A(0)
            if nts > 1:
                emit_A(1)
            bnext = 0
            for j in range(2, nts):
                emit_B_group(4 * (j - 2))
                bnext = 4 * (j - 2) + 4
                emit_A(j)
            for c0 in range(bnext, nch, 4):
                emit_B_group(c0)
            while ntail[0] < nch:
                emit_tail(ntail[0])
                ntail[0] += 1

    nc.compile()
    return nc


def _prep_inputs(inputs, t=T):
    """Per-core input dicts: core = 4*b + h."""
    import ml_dtypes
    bf16 = ml_dtypes.bfloat16
    ins = {k: np.ascontiguousarray(np.asarray(v, dtype=np.float32))
           for k, v in inputs.items()}
    x, Wq, Wk, Wv, Wg = ins["x"], ins["Wq"], ins["Wk"], ins["Wv"], ins["Wg"]
    Wgk12 = (ins["Wgk1"].astype(np.float64) @ ins["Wgk2"].astype(np.float64))
    bgk2, gnorm = ins["bgk2"], ins["gnorm_w"]
    Wo, Whead = ins["Wo"], ins["Whead"]

    um = (np.arange(C)[:, None] <= np.arange(C)[None, :]).astype(bf16)

    def chunk_w(w):  # [512, n] -> [128, 4, n]
        return np.ascontiguousarray(
            w.reshape(4, 128, -1).transpose(1, 0, 2).astype(bf16))

    in_maps = []
    for core in range(8):
        b, h = divmod(core, 4)
        wf = ((gnorm[:, None].astype(np.float64)
               * Wo[h * dv:(h + 1) * dv, :].astype(np.float64))
              @ Whead.astype(np.float64)).astype(np.float32)
        w2 = np.concatenate(
            [Wq[:, h * dk:(h + 1) * dk] * SCALE, Wk[:, h * dk:(h + 1) * dk],
             Wg[:, h * dv:(h + 1) * dv], Wv[:, h * dv:(h + 1) * dv]], axis=1)
        in_maps.append({
            "xt": np.ascontiguousarray(
                x[b, :t].T.reshape(4, 128, t).transpose(1, 0, 2).astype(bf16)),
            "w1": chunk_w(Wgk12[:, h * dk:(h + 1) * dk].astype(np.float32)),
            "w2": chunk_w(w2),
            "umask": um,
            "wfused": np.ascontiguousarray(wf.astype(bf16)),
            "nbgk2": np.ascontiguousarray(-bgk2[h * dk:(h + 1) * dk, None]),
        })
    return in_maps


def _gather(results, inputs, t=T):
    bhead = np.asarray(inputs["bhead"], dtype=np.float32)
    out = np.zeros((B, t, 10), np.float32)
    for core in range(8):
        b = core // 4
        r = results[core]["out10"]          # [128, nch, 10]
        out[b] += r.transpose(1, 0, 2).reshape(t, 10)
    out += bhead[None, None, :]
    return out


def run(inputs, trace=False, **kw):
    from concourse.bass_utils import run_bass_kernel_spmd
    if "nc" not in _CACHE:
        _CACHE["nc"] = build()
    nc = _CACHE["nc"]
    in_maps = _prep_inputs(inputs)
    res = run_bass_kernel_spmd(nc, in_maps, core_ids=list(range(8)),
                               trace=trace, **kw)
    return _gather(res.results, inputs), res


def kernel(**inputs) -> np.ndarray:
    out, _ = run(inputs, trace=False)
    return out


# revision 55
# speedup vs baseline: 1.0469x; 1.0469x over previous
"""Gated linear attention (GLA) Bass kernel for Trainium2, 8 NeuronCores.

Sharding: one core per (batch, head) pair -- B=2 x H=4 = 8 cores.

v2 redesign vs baseline (145us):
  - bf16 throughout: x, weights, chunk matmuls (4x PE rate at 128-col moving
    dim; FWL weight loads; half the HBM traffic for x).
  - transposed-o formulation: po^T[dv,t] = v^T atm + S^T q~ accumulates both
    recurrence terms in one psum group, killing the per-chunk output
    transpose + copy of the baseline.
  - phases pipelined across chunks instead of a serial per-chunk chain:
    B1 (pat/ktn/pds/po-intra, independent per chunk), B2 (state scan
    U_c = U_{c-1}*d_{c-1} + pds_c as one fused vector op reading psum),
    B3 (po += S^T q~), C (rmsnorm via matmul-with-ones + batched Ln/Exp).
  - rmsnorm rstd applied on the 10-col head output instead of the dv-wide
    tensor; sum-of-squares via PE matmul against ones.
  - decay exp computed as ONE [128,512] activation (q|k halves stacked with
    pre-negated sign); softmax scale folded into Wq on host.
  - swish split across scalar/gpsimd/vector; cumsum j>0 on gpsimd.
  - x streamed t-major [128,T,4] so time slices are contiguous DMAs;
    weights packed into 2 DMAs ordered ahead of the x stream.
"""
import sys, os
sys.path.insert(0, "/opt/trn_rl_repo")

import numpy as np

B, T, D = 2, 2048, 512
H = 4
dk, dv = 64, 128          # per-head key/value dims
C = 128                   # chunk length
GATE_NORM = 16.0
EPS = 1e-5
SCALE = dk ** -0.5

_CACHE = {}


def build(t=T):
    import concourse.bass as bass  # noqa: F401
    from concourse import bacc, mybir
    import concourse.tile as tile
    import concourse.hw_specs as hw_specs

    F32 = mybir.dt.float32
    BF16 = mybir.dt.bfloat16
    AF = mybir.ActivationFunctionType
    OP = mybir.AluOpType

    # Keep every activation func we use in one table (see baseline comment):
    # Exp, Ln, Square, Copy, Identity all live in natural_log_exp_and_others.
    need = {AF.Exp, AF.Ln, AF.Square, AF.Copy, AF.Identity}
    keep = "natural_log_exp_and_others"
    tabs = hw_specs.get_activation_tables("gen3")
    if keep in tabs and need <= tabs[keep]:
        for name, s in tabs.items():
            if name != keep:
                s -= need

    nch = t // C              # chunks
    nts = t // 512            # 512-wide time slices
    assert t % 512 == 0 and nch % 4 == 0
    ngrp = nch // 4

    nc = bacc.Bacc("TRN2", target_bir_lowering=False, debug=False)

    xt_d = nc.dram_tensor("xt", [128, 4, t], BF16, kind="ExternalInput")
    w1_d = nc.dram_tensor("w1", [128, 4, dk], BF16, kind="ExternalInput")
    w2_d = nc.dram_tensor("w2", [128, 4, 2 * C + dv], BF16, kind="ExternalInput")
    um_d = nc.dram_tensor("umask", [C, C], BF16, kind="ExternalInput")
    wf_d = nc.dram_tensor("wfused", [dv, 10], BF16, kind="ExternalInput")
    nb_d = nc.dram_tensor("nbgk2", [dk, 1], F32, kind="ExternalInput")
    out_d = nc.dram_tensor("out10", [128, nch, 10], F32, kind="ExternalOutput")

    with tile.TileContext(nc) as tc:
        with (
            tc.tile_pool(name="wt", bufs=1) as wt,
            tc.tile_pool(name="sm", bufs=2) as sm,
            tc.tile_pool(name="ck", bufs=3) as ck,
            tc.tile_pool(name="am", bufs=10) as am,
            tc.tile_pool(name="pp", bufs=4, space="PSUM") as pp,
            tc.tile_pool(name="pq", bufs=1, space="PSUM") as pq,
        ):
            # ---- persistent SBUF ----
            xt = wt.tile([128, 4, t], BF16)
            w1 = wt.tile([128, 4, dk], BF16)           # wgk (low-rank fused)
            w2 = wt.tile([128, 4, 2 * C + dv], BF16)   # [wqk 128 | wg 128 | wv 128]
            umb = wt.tile([C, C], BF16)
            wf_sb = wt.tile([dv, 10], BF16)
            nb_sb = wt.tile([dk, 1], F32)

            # input DMAs: weights ahead of the x stream on the sync queue.
            # first 512-slice split into 4 pieces so slice-0 projections can
            # start on partial data.
            nc.scalar.dma_start(w1[:], w1_d[:])
            for p in range(2):
                nc.sync.dma_start(xt[:, :, p * 256:(p + 1) * 256],
                                  xt_d[:, :, p * 256:(p + 1) * 256])
            nc.sync.dma_start(w2[:], w2_d[:])
            for j in range(1, nts):
                nc.sync.dma_start(xt[:, :, j * 512:(j + 1) * 512],
                                  xt_d[:, :, j * 512:(j + 1) * 512])
            nc.gpsimd.dma_start(umb[:], um_d[:])
            nc.gpsimd.dma_start(wf_sb[:], wf_d[:])
            nc.gpsimd.dma_start(nb_sb[:], nb_d[:])

            wqk = w2[:, :, 0:C]
            wg = w2[:, :, C:2 * C]
            wv = w2[:, :, 2 * C:2 * C + dv]

            ones64 = wt.tile([dk, 1], F32)
            nc.vector.memset(ones64[:], 1.0)
            onesbf = wt.tile([128, 1], BF16)
            nc.vector.memset(onesbf[:], 1.0)
            eps_sb = wt.tile([128, 1], F32)
            nc.vector.memset(eps_sb[:], EPS)
            # scan reset mask: 0 at chunk starts
            mres = wt.tile([dk, 512], F32)
            nc.vector.memset(mres[:], 1.0)
            mres_v = mres[:].rearrange("p (c l) -> p c l", l=C)
            nc.vector.memset(mres_v[:, :, 0:1], 0.0)

            # big SBUF activations
            spc = wt.tile([dk, t], F32)       # per-chunk cumsum of softplus
            qt_t = wt.tile([dk, t], BF16)     # q~^T
            kt_t = wt.tile([dk, t], BF16)     # k~^T
            swt = wt.tile([dv, t], BF16)      # swish(g)^T
            vsb = wt.tile([128, nch, dv], BF16)
            dlast = wt.tile([dk, nch], F32)
            ktn = wt.tile([C, nch, dk], BF16)
            Sb = wt.tile([dk, nch, dv], BF16)
            obuf = wt.tile([128, nch, 10], F32)

            dl_src = spc[:].rearrange("p (c l) -> p c l", l=C)

            # ---- PSUM (bank-granular: 4 + 1 + 1 + 1 = 7 of 8 banks) ----
            prs = pq.tile([128, 512], F32)    # pat 2 slots | pds 2 slots
            pvt = pq.tile([128, 4, C], F32)   # odd po^T slots
            pot = pq.tile([128, 4, C], F32)   # even po^T slots
            pt2 = pq.tile([128, nch + nch * 10], F32)  # ssq | p10 cols

            def pat_s(c):
                return prs[:, (c % 2) * C:(c % 2) * C + C]

            def pds_s(c):
                return prs[0:dk, 256 + (c % 2) * C:256 + (c % 2) * C + C]

            def ssq_s(c):
                return pt2[:, c:c + 1]

            def p10_s(c):
                return pt2[:, nch + c * 10:nch + (c + 1) * 10]

            def sl(c):
                return slice(c * C, (c + 1) * C)

            # ---------------- phase A: projections ----------------
            def emit_A(j):
                ts = slice(j * 512, (j + 1) * 512)
                xs = xt[:, :, ts]
                # j=0 runs piecewise so matmuls start as soon as the first
                # 256-col DMA piece lands
                pieces = ([slice(p * 256, (p + 1) * 256) for p in range(2)]
                          if j == 0 else [slice(0, 512)])

                def proj(ps, w_sb):
                    for pr in pieces:
                        for d4 in range(4):
                            nc.tensor.matmul(ps[:, pr], w_sb[:, d4, :],
                                             xs[:, d4, pr],
                                             start=(d4 == 0), stop=(d4 == 3))

                # gate chain
                pz = pp.tile([dk, 512], F32, tag="P")
                proj(pz, w1)
                eg = sm.tile([dk, 512], BF16, tag="eg")
                nc.scalar.activation(out=eg[:], in_=pz[:], func=AF.Exp,
                                     scale=-1.0, bias=nb_sb[:])
                sp = sm.tile([dk, 512], F32, tag="sp")
                nc.scalar.activation(out=sp[:], in_=eg[:], func=AF.Ln,
                                     bias=ones64[:])
                nc.vector.tensor_tensor_scan(
                    out=spc[:, ts], data0=mres[:], data1=sp[:],
                    initial=0.0, op0=OP.mult, op1=OP.add)
                nc.scalar.activation(out=dlast[:, 4 * j:4 * j + 4],
                                     in_=dl_src[:, 4 * j:4 * j + 4, C - 1:C],
                                     func=AF.Exp, scale=-1.0 / GATE_NORM)
                eeq = sm.tile([dk, 512], F32, tag="eeq")
                nc.scalar.activation(out=eeq[:], in_=spc[:, ts], func=AF.Exp,
                                     scale=-1.0 / GATE_NORM)
                eek = sm.tile([dk, 512], F32, tag="eek")
                nc.scalar.activation(out=eek[:], in_=spc[:, ts], func=AF.Exp,
                                     scale=1.0 / GATE_NORM)

                # q|k projection + decay
                pqk = pp.tile([128, 512], F32, tag="P")
                proj(pqk, wqk)
                nc.vector.tensor_tensor(out=qt_t[:, ts], in0=pqk[0:dk, :],
                                        in1=eeq[:], op=OP.mult)
                nc.vector.tensor_tensor(out=kt_t[:, ts], in0=pqk[64:128, :],
                                        in1=eek[:], op=OP.mult)
                # k^T chunk transposes on the (idle) DMA xbar: [64,512] ->
                # [128, 4, 64] blocked per chunk, ahead of the phase-B scan
                nc.sync.dma_start_transpose(ktn[:, 4 * j:4 * j + 4, :],
                                            kt_t[:, ts])


                # g^T projection + swish
                pgt = pp.tile([128, 512], F32, tag="P")
                proj(pgt, wg)
                eg2 = sm.tile([dv, 512], BF16, tag="eg2")
                nc.scalar.activation(out=eg2[:], in_=pgt[:], func=AF.Exp,
                                     scale=-1.0)
                s1 = sm.tile([dv, 512], F32, tag="s1")
                nc.vector.tensor_scalar_add(out=s1[:], in0=eg2[:], scalar1=1.0)
                s2 = sm.tile([dv, 512], F32, tag="s2")
                nc.vector.reciprocal_approx_fast(out=s2[:], in_=s1[:])
                nc.vector.tensor_tensor(out=swt[:, ts], in0=pgt[:], in1=s2[:],
                                        op=OP.mult)

                # v projection wv-stationary (4 LDWs/slice instead of 64):
                # v^T psum -> bf16 -> blocked DMA transpose to natural layout
                pvT = pp.tile([dv, 512], F32, tag="P")
                proj(pvT, wv)
                vtb = sm.tile([dv, 512], BF16, tag="vtb")
                if j % 2 == 0:
                    nc.scalar.copy(vtb[:], pvT[:])
                else:
                    nc.vector.tensor_copy(vtb[:], pvT[:])
                nc.sync.dma_start_transpose(vsb[:, 4 * j:4 * j + 4, :],
                                            vtb[:])

            # ---------------- phase B/C: chunked recurrence ----------------
            U = [None, None]

            def po_s(c):
                # alternate po groups between two banks so consecutive
                # accumulation groups never contend for the same psum bank
                bank = pot if c % 2 == 0 else pvt
                return bank[:, (c // 2) % 4, :]

            def emit_c(c):
                """post-processing of chunk c (po complete)."""
                po = po_s(c)
                ot = ck.tile([dv, C], BF16, tag="ot")
                nc.vector.tensor_tensor(out=ot[:], in0=po, in1=swt[:, sl(c)],
                                        op=OP.mult)
                sq = ck.tile([dv, C], BF16, tag="sq")
                nc.scalar.activation(out=sq[:], in_=po, func=AF.Square)
                nc.tensor.matmul(ssq_s(c), sq[:], onesbf[:],
                                 start=True, stop=True)
                nc.tensor.matmul(p10_s(c), ot[:], wf_sb[:],
                                 start=True, stop=True)
                if c % 4 == 3:
                    g = c // 4
                    lnv = ck.tile([128, 4], F32, tag="lnv")
                    nc.scalar.activation(out=lnv[:], in_=pt2[:, 4 * g:4 * g + 4],
                                         func=AF.Ln, scale=1.0 / dv,
                                         bias=eps_sb[:])
                    rstd = ck.tile([128, 4], F32, tag="rstd")
                    nc.scalar.activation(out=rstd[:], in_=lnv[:], func=AF.Exp,
                                         scale=-0.5)
                    p10g = pt2[:, nch + g * 40:nch + (g + 1) * 40]
                    nc.vector.tensor_tensor(
                        out=obuf[:, 4 * g:4 * g + 4, :],
                        in0=p10g.rearrange("p (c n) -> p c n", n=10),
                        in1=rstd[:].unsqueeze(2).broadcast_to([128, 4, 10]),
                        op=OP.mult)
                    nc.sync.dma_start(out_d[:, 4 * g:4 * g + 4, :],
                                      obuf[:, 4 * g:4 * g + 4, :])

            LAG = min(8, nch - 1)
            atms = [None] * nch

            def emit_tail(x):
                po = po_s(x)
                if x > 0:
                    nc.tensor.matmul(po, Sb[:, x - 1, :], qt_t[:, sl(x)],
                                     start=True, stop=False)
                nc.tensor.matmul(po, vsb[:, x, :], atms[x][:],
                                 start=(x == 0), stop=True)
                emit_c(x)

            def emit_B(c):
                cs = sl(c)
                qt_c = qt_t[:, cs]
                kt_c = kt_t[:, cs]
                v_c = vsb[:, c, :]
                # B1
                pat = pat_s(c)
                nc.tensor.matmul(pat, kt_c, qt_c, start=True, stop=True)
                # pat evict on scalar, causal mask on gpsimd - keeps the
                # loaded vector queue out of this path entirely
                patr = am.tile([C, C], BF16, tag="patr")
                nc.scalar.copy(patr[:], pat)
                atm = am.tile([C, C], BF16, tag="atm")
                atms[c] = atm
                nc.gpsimd.tensor_tensor(out=atm[:], in0=patr[:], in1=umb[:],
                                        op=OP.mult)
                pds = pds_s(c)
                nc.tensor.matmul(pds, ktn[:, c, :], v_c, start=True, stop=True)
                # B2: U_c = U_{c-1} * d_{c-1} + pds_c ; Sb_c = bf16(U_c * d_c)
                Uc = ck.tile([dk, dv], F32, tag="U")
                if c == 0:
                    nc.vector.tensor_copy(Uc[:], pds)
                else:
                    nc.vector.scalar_tensor_tensor(
                        out=Uc[:], in0=U[(c - 1) % 2][:],
                        scalar=dlast[:, c - 1:c], op0=OP.mult,
                        in1=pds, op1=OP.add)
                U[c % 2] = Uc
                if c < nch - 1:
                    nc.gpsimd.tensor_tensor(
                        out=Sb[:, c, :], in0=Uc[:],
                        in1=dlast[:, c:c + 1].broadcast_to([dk, dv]),
                        op=OP.mult)

            # ------------- interleaved schedule -------------
            # A-slices and B-chunk-groups interleave so phase-B work fills
            # phase-A queue gaps; chunk tails trail by LAG behind the scan.
            ntail = [0]

            def emit_B_group(c0):
                for c in range(c0, c0 + 4):
                    emit_B(c)
                    while ntail[0] <= c - LAG:
                        emit_tail(ntail[0])
                        ntail[0] += 1

            emit_A(0)
            if nts > 1:
                emit_A(1)
            bnext = 0
            for j in range(2, nts):
                emit_B_group(4 * (j - 2))
                bnext = 4 * (j - 2) + 4
                emit_A(j)
            for c0 in range(bnext, nch, 4):
                emit_B_group(c0)
            while ntail[0] < nch:
                emit_tail(ntail[0])
                ntail[0] += 1

    nc.compile()
    return nc


def _prep_inputs(inputs, t=T):
    """Per-core input dicts: core = 4*b + h."""
    import ml_dtypes
    bf16 = ml_dtypes.bfloat16
    ins = {k: np.ascontiguousarray(np.asarray(v, dtype=np.float32))
           for k, v in inputs.items()}
    x, Wq, Wk, Wv, Wg = ins["x"], ins["Wq"], ins["Wk"], ins["Wv"], ins["Wg"]
    Wgk12 = (ins["Wgk1"].astype(np.float64) @ ins["Wgk2"].astype(np.float64))
    bgk2, gnorm = ins["bgk2"], ins["gnorm_w"]
    Wo, Whead = ins["Wo"], ins["Whead"]

    um = (np.arange(C)[:, None] <= np.arange(C)[None, :]).astype(bf16)

    def chunk_w(w):  # [512, n] -> [128, 4, n]
        return np.ascontiguousarray(
            w.reshape(4, 128, -1).transpose(1, 0, 2).astype(bf16))

    in_maps = []
    for core in range(8):
        b, h = divmod(core, 4)
        wf = ((gnorm[:, None].astype(np.float64)
               * Wo[h * dv:(h + 1) * dv, :].astype(np.float64))
              @ Whead.astype(np.float64)).astype(np.float32)
        w2 = np.concatenate(
            [Wq[:, h * dk:(h + 1) * dk] * SCALE, Wk[:, h * dk:(h + 1) * dk],
             Wg[:, h * dv:(h + 1) * dv], Wv[:, h * dv:(h + 1) * dv]], axis=1)
        in_maps.append({
            "xt": np.ascontiguousarray(
                x[b, :t].T.reshape(4, 128, t).transpose(1, 0, 2).astype(bf16)),
            "w1": chunk_w(Wgk12[:, h * dk:(h + 1) * dk].astype(np.float32)),
            "w2": chunk_w(w2),
            "umask": um,
            "wfused": np.ascontiguousarray(wf.astype(bf16)),
            "nbgk2": np.ascontiguousarray(-bgk2[h * dk:(h + 1) * dk, None]),
        })
    return in_maps


def _gather(results, inputs, t=T):
    bhead = np.asarray(inputs["bhead"], dtype=np.float32)
    out = np.zeros((B, t, 10), np.float32)
    for core in range(8):
        b = core // 4
        r = results[core]["out10"]          # [128, nch, 10]
        out[b] += r.transpose(1, 0, 2).reshape(t, 10)
    out += bhead[None, None, :]
    return out


def run(inputs, trace=False, **kw):
    from concourse.bass_utils import run_bass_kernel_spmd
    if "nc" not in _CACHE:
        _CACHE["nc"] = build()
    nc = _CACHE["nc"]
    in_maps = _prep_inputs(inputs)
    res = run_bass_kernel_spmd(nc, in_maps, core_ids=list(range(8)),
                               trace=trace, **kw)
    return _gather(res.results, inputs), res


def kernel(**inputs) -> np.ndarray:
    out, _ = run(inputs, trace=False)
    return out


# revision 56
# speedup vs baseline: 1.1412x; 1.0901x over previous
"""Gated linear attention (GLA) Bass kernel for Trainium2, 8 NeuronCores.

Sharding: one core per (batch, head) pair -- B=2 x H=4 = 8 cores.

v2 redesign vs baseline (145us):
  - bf16 throughout: x, weights, chunk matmuls (4x PE rate at 128-col moving
    dim; FWL weight loads; half the HBM traffic for x).
  - transposed-o formulation: po^T[dv,t] = v^T atm + S^T q~ accumulates both
    recurrence terms in one psum group, killing the per-chunk output
    transpose + copy of the baseline.
  - phases pipelined across chunks instead of a serial per-chunk chain:
    B1 (pat/ktn/pds/po-intra, independent per chunk), B2 (state scan
    U_c = U_{c-1}*d_{c-1} + pds_c as one fused vector op reading psum),
    B3 (po += S^T q~), C (rmsnorm via matmul-with-ones + batched Ln/Exp).
  - rmsnorm rstd applied on the 10-col head output instead of the dv-wide
    tensor; sum-of-squares via PE matmul against ones.
  - decay exp computed as ONE [128,512] activation (q|k halves stacked with
    pre-negated sign); softmax scale folded into Wq on host.
  - swish split across scalar/gpsimd/vector; cumsum j>0 on gpsimd.
  - x streamed t-major [128,T,4] so time slices are contiguous DMAs;
    weights packed into 2 DMAs ordered ahead of the x stream.
"""
import sys, os
sys.path.insert(0, "/opt/trn_rl_repo")

import numpy as np

B, T, D = 2, 2048, 512
H = 4
dk, dv = 64, 128          # per-head key/value dims
C = 128                   # chunk length
GATE_NORM = 16.0
EPS = 1e-5
SCALE = dk ** -0.5

_CACHE = {}


def build(t=T):
    import concourse.bass as bass  # noqa: F401
    from concourse import bacc, mybir
    import concourse.tile as tile
    import concourse.hw_specs as hw_specs

    F32 = mybir.dt.float32
    BF16 = mybir.dt.bfloat16
    AF = mybir.ActivationFunctionType
    OP = mybir.AluOpType

    # Keep every activation func we use in one table (see baseline comment):
    # Exp, Ln, Square, Copy, Identity all live in natural_log_exp_and_others.
    need = {AF.Exp, AF.Ln, AF.Square, AF.Copy, AF.Identity}
    keep = "natural_log_exp_and_others"
    tabs = hw_specs.get_activation_tables("gen3")
    if keep in tabs and need <= tabs[keep]:
        for name, s in tabs.items():
            if name != keep:
                s -= need

    nch = t // C              # chunks
    nts = t // 512            # 512-wide time slices
    assert t % 512 == 0 and nch % 4 == 0
    ngrp = nch // 4

    nc = bacc.Bacc("TRN2", target_bir_lowering=False, debug=False)

    xt_d = nc.dram_tensor("xt", [128, 4, t], BF16, kind="ExternalInput")
    w1_d = nc.dram_tensor("w1", [128, 4, dk], BF16, kind="ExternalInput")
    w2_d = nc.dram_tensor("w2", [128, 4, 2 * C + dv], BF16, kind="ExternalInput")
    um_d = nc.dram_tensor("umask", [C, C], BF16, kind="ExternalInput")
    wf_d = nc.dram_tensor("wfused", [dv, 10], BF16, kind="ExternalInput")
    nb_d = nc.dram_tensor("nbgk2", [dk, 1], F32, kind="ExternalInput")
    out_d = nc.dram_tensor("out10", [128, nch, 10], F32, kind="ExternalOutput")

    with tile.TileContext(nc) as tc:
        with (
            tc.tile_pool(name="wt", bufs=1) as wt,
            tc.tile_pool(name="sm", bufs=2) as sm,
            tc.tile_pool(name="ck", bufs=3) as ck,
            tc.tile_pool(name="am", bufs=10) as am,
            tc.tile_pool(name="pp", bufs=4, space="PSUM") as pp,
            tc.tile_pool(name="pq", bufs=1, space="PSUM") as pq,
        ):
            # ---- persistent SBUF ----
            xt = wt.tile([128, 4, t], BF16)
            w1 = wt.tile([128, 4, dk], BF16)           # wgk (low-rank fused)
            w2 = wt.tile([128, 4, 2 * C + dv], BF16)   # [wqk 128 | wg 128 | wv 128]
            umb = wt.tile([C, C], BF16)
            wf_sb = wt.tile([dv, 10], BF16)
            nb_sb = wt.tile([dk, 1], F32)

            # input DMAs: weights ahead of the x stream on the sync queue.
            # first 512-slice split into 4 pieces so slice-0 projections can
            # start on partial data.
            nc.scalar.dma_start(w1[:], w1_d[:])
            for p in range(2):
                nc.sync.dma_start(xt[:, :, p * 256:(p + 1) * 256],
                                  xt_d[:, :, p * 256:(p + 1) * 256])
            nc.sync.dma_start(w2[:], w2_d[:])
            for j in range(1, nts):
                nc.sync.dma_start(xt[:, :, j * 512:(j + 1) * 512],
                                  xt_d[:, :, j * 512:(j + 1) * 512])
            nc.gpsimd.dma_start(umb[:], um_d[:])
            nc.gpsimd.dma_start(wf_sb[:], wf_d[:])
            nc.gpsimd.dma_start(nb_sb[:], nb_d[:])

            wqk = w2[:, :, 0:C]
            wg = w2[:, :, C:2 * C]
            wv = w2[:, :, 2 * C:2 * C + dv]

            ones64 = wt.tile([dk, 1], F32)
            nc.vector.memset(ones64[:], 1.0)
            onesbf = wt.tile([128, 1], BF16)
            nc.vector.memset(onesbf[:], 1.0)
            eps_sb = wt.tile([128, 1], F32)
            nc.vector.memset(eps_sb[:], EPS)
            # scan reset mask: 0 at chunk starts
            mres = wt.tile([dk, 512], F32)
            nc.vector.memset(mres[:], 1.0)
            mres_v = mres[:].rearrange("p (c l) -> p c l", l=C)
            nc.vector.memset(mres_v[:, :, 0:1], 0.0)

            # big SBUF activations
            spc = wt.tile([dk, t], F32)       # per-chunk cumsum of softplus
            qt_t = wt.tile([dk, t], BF16)     # q~^T
            kt_t = wt.tile([dk, t], BF16)     # k~^T
            swt = wt.tile([dv, t], BF16)      # swish(g)^T
            vsb = wt.tile([128, nch, dv], BF16)
            dlast = wt.tile([dk, nch], F32)
            ktn = wt.tile([C, nch, dk], BF16)
            Sb = wt.tile([dk, nch, dv], BF16)
            obuf = wt.tile([128, nch, 10], F32)

            dl_src = spc[:].rearrange("p (c l) -> p c l", l=C)

            # ---- PSUM (bank-granular: 4 + 1 + 1 + 1 = 7 of 8 banks) ----
            prs = pq.tile([128, 512], F32)    # pat 2 slots | pds 2 slots
            pvt = pq.tile([128, 4, C], F32)   # odd po^T slots
            pot = pq.tile([128, 4, C], F32)   # even po^T slots
            pt2 = pq.tile([128, nch + nch * 10], F32)  # ssq | p10 cols

            def pat_s(c):
                return prs[:, (c % 2) * C:(c % 2) * C + C]

            def pds_s(c):
                return prs[0:dk, 256 + (c % 2) * C:256 + (c % 2) * C + C]

            def ssq_s(c):
                return pt2[:, c:c + 1]

            def p10_s(c):
                return pt2[:, nch + c * 10:nch + (c + 1) * 10]

            def sl(c):
                return slice(c * C, (c + 1) * C)

            # ---------------- phase A: projections ----------------
            def emit_A(j):
                ts = slice(j * 512, (j + 1) * 512)
                xs = xt[:, :, ts]
                # j=0 runs piecewise so matmuls start as soon as the first
                # 256-col DMA piece lands
                pieces = ([slice(p * 256, (p + 1) * 256) for p in range(2)]
                          if j == 0 else [slice(0, 512)])

                def proj(ps, w_sb):
                    for pr in pieces:
                        for d4 in range(4):
                            nc.tensor.matmul(ps[:, pr], w_sb[:, d4, :],
                                             xs[:, d4, pr],
                                             start=(d4 == 0), stop=(d4 == 3))

                # gate chain
                pz = pp.tile([dk, 512], F32, tag="P")
                proj(pz, w1)
                eg = sm.tile([dk, 512], BF16, tag="eg")
                nc.scalar.activation(out=eg[:], in_=pz[:], func=AF.Exp,
                                     scale=-1.0, bias=nb_sb[:])
                sp = sm.tile([dk, 512], F32, tag="sp")
                nc.scalar.activation(out=sp[:], in_=eg[:], func=AF.Ln,
                                     bias=ones64[:])
                nc.vector.tensor_tensor_scan(
                    out=spc[:, ts], data0=mres[:], data1=sp[:],
                    initial=0.0, op0=OP.mult, op1=OP.add)
                nc.scalar.activation(out=dlast[:, 4 * j:4 * j + 4],
                                     in_=dl_src[:, 4 * j:4 * j + 4, C - 1:C],
                                     func=AF.Exp, scale=-1.0 / GATE_NORM)
                eeq = sm.tile([dk, 512], F32, tag="eeq")
                nc.scalar.activation(out=eeq[:], in_=spc[:, ts], func=AF.Exp,
                                     scale=-1.0 / GATE_NORM)
                eek = sm.tile([dk, 512], F32, tag="eek")
                nc.scalar.activation(out=eek[:], in_=spc[:, ts], func=AF.Exp,
                                     scale=1.0 / GATE_NORM)

                # q|k projection + decay
                pqk = pp.tile([128, 512], F32, tag="P")
                proj(pqk, wqk)
                nc.vector.tensor_tensor(out=qt_t[:, ts], in0=pqk[0:dk, :],
                                        in1=eeq[:], op=OP.mult)
                nc.vector.tensor_tensor(out=kt_t[:, ts], in0=pqk[64:128, :],
                                        in1=eek[:], op=OP.mult)
                # k^T chunk transposes on the (idle) DMA xbar: [64,512] ->
                # [128, 4, 64] blocked per chunk, ahead of the phase-B scan
                nc.sync.dma_start_transpose(ktn[:, 4 * j:4 * j + 4, :],
                                            kt_t[:, ts])


                # g^T projection + swish
                pgt = pp.tile([128, 512], F32, tag="P")
                proj(pgt, wg)
                eg2 = sm.tile([dv, 512], BF16, tag="eg2")
                nc.scalar.activation(out=eg2[:], in_=pgt[:], func=AF.Exp,
                                     scale=-1.0)
                s1 = sm.tile([dv, 512], F32, tag="s1")
                nc.vector.tensor_scalar_add(out=s1[:], in0=eg2[:], scalar1=1.0)
                s2 = sm.tile([dv, 512], F32, tag="s2")
                nc.vector.reciprocal_approx_fast(out=s2[:], in_=s1[:])
                nc.vector.tensor_tensor(out=swt[:, ts], in0=pgt[:], in1=s2[:],
                                        op=OP.mult)

                # v projection wv-stationary (4 LDWs/slice instead of 64):
                # v^T psum -> bf16 -> blocked DMA transpose to natural layout
                pvT = pp.tile([dv, 512], F32, tag="P")
                proj(pvT, wv)
                vtb = sm.tile([dv, 512], BF16, tag="vtb")
                if j % 2 == 0:
                    nc.scalar.copy(vtb[:], pvT[:])
                else:
                    nc.vector.tensor_copy(vtb[:], pvT[:])
                nc.sync.dma_start_transpose(vsb[:, 4 * j:4 * j + 4, :],
                                            vtb[:])

            # ---------------- phase B/C: chunked recurrence ----------------
            U = [None, None]

            def po_s(c):
                # alternate po groups between two banks so consecutive
                # accumulation groups never contend for the same psum bank
                bank = pot if c % 2 == 0 else pvt
                return bank[:, (c // 2) % 4, :]

            def emit_c(c):
                """post-processing of chunk c (po complete)."""
                po = po_s(c)
                ot = ck.tile([dv, C], BF16, tag="ot")
                nc.vector.tensor_tensor(out=ot[:], in0=po, in1=swt[:, sl(c)],
                                        op=OP.mult)
                sq = ck.tile([dv, C], BF16, tag="sq")
                nc.scalar.activation(out=sq[:], in_=po, func=AF.Square)
                nc.tensor.matmul(ssq_s(c), sq[:], onesbf[:],
                                 start=True, stop=True)
                nc.tensor.matmul(p10_s(c), ot[:], wf_sb[:],
                                 start=True, stop=True)
                if c % 4 == 3:
                    g = c // 4
                    lnv = ck.tile([128, 4], F32, tag="lnv")
                    nc.scalar.activation(out=lnv[:], in_=pt2[:, 4 * g:4 * g + 4],
                                         func=AF.Ln, scale=1.0 / dv,
                                         bias=eps_sb[:])
                    rstd = ck.tile([128, 4], F32, tag="rstd")
                    nc.scalar.activation(out=rstd[:], in_=lnv[:], func=AF.Exp,
                                         scale=-0.5)
                    p10g = pt2[:, nch + g * 40:nch + (g + 1) * 40]
                    nc.vector.tensor_tensor(
                        out=obuf[:, 4 * g:4 * g + 4, :],
                        in0=p10g.rearrange("p (c n) -> p c n", n=10),
                        in1=rstd[:].unsqueeze(2).broadcast_to([128, 4, 10]),
                        op=OP.mult)
                    nc.sync.dma_start(out_d[:, 4 * g:4 * g + 4, :],
                                      obuf[:, 4 * g:4 * g + 4, :])

            LAG = min(8, nch - 1)
            atms = [None] * nch

            def emit_tail(x):
                po = po_s(x)
                if x > 0:
                    nc.tensor.matmul(po, Sb[:, x - 1, :], qt_t[:, sl(x)],
                                     start=True, stop=False)
                nc.tensor.matmul(po, vsb[:, x, :], atms[x],
                                 start=(x == 0), stop=True)
                emit_c(x)

            def emit_B(c):
                cs = sl(c)
                qt_c = qt_t[:, cs]
                kt_c = kt_t[:, cs]
                v_c = vsb[:, c, :]
                # B1
                pat = pat_s(c)
                nc.tensor.matmul(pat, kt_c, qt_c, start=True, stop=True)
                # pat evict on scalar + causal mask on gpsimd, batched in
                # PAIRS of chunks (the two pat slots are adjacent columns of
                # one psum bank) to halve the fixed per-op overhead
                if c % 2 == 1:
                    patr = am.tile([C, 2, C], BF16, tag="patr")
                    nc.scalar.copy(patr[:], prs[:, 0:2 * C])
                    atm2 = am.tile([C, 2, C], BF16, tag="atm")
                    nc.gpsimd.tensor_tensor(
                        out=atm2[:], in0=patr[:],
                        in1=umb[:].unsqueeze(1).broadcast_to([C, 2, C]),
                        op=OP.mult)
                    atms[c - 1] = atm2[:, 0, :]
                    atms[c] = atm2[:, 1, :]
                pds = pds_s(c)
                nc.tensor.matmul(pds, ktn[:, c, :], v_c, start=True, stop=True)
                # B2: U_c = U_{c-1} * d_{c-1} + pds_c ; Sb_c = bf16(U_c * d_c)
                Uc = ck.tile([dk, dv], F32, tag="U")
                if c == 0:
                    nc.vector.tensor_copy(Uc[:], pds)
                else:
                    nc.vector.scalar_tensor_tensor(
                        out=Uc[:], in0=U[(c - 1) % 2][:],
                        scalar=dlast[:, c - 1:c], op0=OP.mult,
                        in1=pds, op1=OP.add)
                U[c % 2] = Uc
                if c < nch - 1:
                    nc.gpsimd.tensor_tensor(
                        out=Sb[:, c, :], in0=Uc[:],
                        in1=dlast[:, c:c + 1].broadcast_to([dk, dv]),
                        op=OP.mult)

            # ------------- interleaved schedule -------------
            # A-slices and B-chunk-groups interleave so phase-B work fills
            # phase-A queue gaps; chunk tails trail by LAG behind the scan.
            ntail = [0]

            def emit_B_group(c0):
                for c in range(c0, c0 + 4):
                    emit_B(c)
                    while ntail[0] <= c - LAG:
                        emit_tail(ntail[0])
                        ntail[0] += 1

            emit_A(0)
            if nts > 1:
                emit_A(1)
            bnext = 0
            for j in range(2, nts):
                emit_B_group(4 * (j - 2))
                bnext = 4 * (j - 2) + 4
                emit_A(j)
            for c0 in range(bnext, nch, 4):
                emit_B_group(c0)
            while ntail[0] < nch:
                emit_tail(ntail[0])
                ntail[0] += 1

    nc.compile()
    return nc


def _prep_inputs(inputs, t=T):
    """Per-core input dicts: core = 4*b + h."""
    import ml_dtypes
    bf16 = ml_dtypes.bfloat16
    ins = {k: np.ascontiguousarray(np.asarray(v, dtype=np.float32))
           for k, v in inputs.items()}
    x, Wq, Wk, Wv, Wg = ins["x"], ins["Wq"], ins["Wk"], ins["Wv"], ins["Wg"]
    Wgk12 = (ins["Wgk1"].astype(np.float64) @ ins["Wgk2"].astype(np.float64))
    bgk2, gnorm = ins["bgk2"], ins["gnorm_w"]
    Wo, Whead = ins["Wo"], ins["Whead"]

    um = (np.arange(C)[:, None] <= np.arange(C)[None, :]).astype(bf16)

    def chunk_w(w):  # [512, n] -> [128, 4, n]
        return np.ascontiguousarray(
            w.reshape(4, 128, -1).transpose(1, 0, 2).astype(bf16))

    in_maps = []
    for core in range(8):
        b, h = divmod(core, 4)
        wf = ((gnorm[:, None].astype(np.float64)
               * Wo[h * dv:(h + 1) * dv, :].astype(np.float64))
              @ Whead.astype(np.float64)).astype(np.float32)
        w2 = np.concatenate(
            [Wq[:, h * dk:(h + 1) * dk] * SCALE, Wk[:, h * dk:(h + 1) * dk],
             Wg[:, h * dv:(h + 1) * dv], Wv[:, h * dv:(h + 1) * dv]], axis=1)
        in_maps.append({
            "xt": np.ascontiguousarray(
                x[b, :t].T.reshape(4, 128, t).transpose(1, 0, 2).astype(bf16)),
            "w1": chunk_w(Wgk12[:, h * dk:(h + 1) * dk].astype(np.float32)),
            "w2": chunk_w(w2),
            "umask": um,
            "wfused": np.ascontiguousarray(wf.astype(bf16)),
            "nbgk2": np.ascontiguousarray(-bgk2[h * dk:(h + 1) * dk, None]),
        })
    return in_maps


def _gather(results, inputs, t=T):
    bhead = np.asarray(inputs["bhead"], dtype=np.float32)
    out = np.zeros((B, t, 10), np.float32)
    for core in range(8):
        b = core // 4
        r = results[core]["out10"]          # [128, nch, 10]
        out[b] += r.transpose(1, 0, 2).reshape(t, 10)
    out += bhead[None, None, :]
    return out


def run(inputs, trace=False, **kw):
    from concourse.bass_utils import run_bass_kernel_spmd
    if "nc" not in _CACHE:
        _CACHE["nc"] = build()
    nc = _CACHE["nc"]
    in_maps = _prep_inputs(inputs)
    res = run_bass_kernel_spmd(nc, in_maps, core_ids=list(range(8)),
                               trace=trace, **kw)
    return _gather(res.results, inputs), res


def kernel(**inputs) -> np.ndarray:
    out, _ = run(inputs, trace=False)
    return out
